# revision 9
# baseline (speedup 1.0000x reference)
"""Distributed Trainium2 Bass kernel for nn_Attention_74732430950409.

Single-query MHA with RoPE'd keys/values; the four projections on the
query side are folded algebraically onto the (1 x d) query:

  qtil[h,:] = (((x @ Wq.T) @ Wq_mha.T)[h] @ Wk_mha[h]) @ Wk        (16, 2048)
  logits[s,h] = rope(keys)[s,:] . qtil[h,:] / sqrt(128)
  w = exp(logits)          (no max subtraction; |logits| small)
  u[h,:] = sum_s w[s,h] * rope(states)[s,:]                        (16, 2048)
  l[h]   = sum_s w[s,h]
  z[h,:]  = (u[h,:] @ Wv.T) / l[h]                                 (16, 2048)
  attn[h,:] = z[h,:] @ Wv_mha[h].T                                 (16, 128)
  out = attn.flat @ Wo.T + x

Sequence-sharded over 8 cores (1024 rows each); weights row-sharded
(256 rows each).  Four AllReduces: qh, qtilT, u|l, attnT.

Performance structure (vs the first working version):
  - All bulk input DMA rides the sync-engine HWDGE queue in strict
    priority order; host pre-tiles every tensor into [128, ...] layout
    so each descriptor is 2-8KB contiguous.
  - Collective bounce buffers + small reads use the gpsimd SW-DGE
    queue so AllReduces trigger as soon as their inputs exist instead
    of queueing behind the bulk stream.
  - RoPE is expressed as 4 elementwise products per tile pair; the
    combining add/sub is folded into the PE's PSUM accumulation of the
    logits / u GEMMs (negated copies of qtil / wT provide the signs).
  - States-side products are split across vector and gpsimd engines.
Compute dtype bf16 (f32 PSUM accumulation).
"""

import sys
import numpy as np

for p in ("/opt/trn_rl_repo",):
    if p not in sys.path:
        sys.path.insert(0, p)

import ml_dtypes

BF16 = ml_dtypes.bfloat16
F8E3 = ml_dtypes.float8_e3m4
WSCALE = 64.0

NUM_HEADS = 16
QK = 2048
VO = 2048
S = 8192
NC = 8
S_LOC = S // NC          # 1024
SH = VO // NC            # 256 rows per core of each weight
DQ = QK // NUM_HEADS     # 128
HALF = VO // 2           # 1024
ROPE_THETA = 10000.0

_cache = {}


def _build():
    import concourse.bass as bass
    import concourse.mybir as mybir
    import concourse.bacc as bacc
    import concourse.tile as tile

    f32 = mybir.dt.float32
    bf16 = mybir.dt.bfloat16
    fp8 = mybir.dt.float8e3
    AF = mybir.ActivationFunctionType
    ALU = mybir.AluOpType
    PSUM = bass.MemorySpace.PSUM

    nc = bacc.Bacc(None, target_bir_lowering=False)

    # ---------------- DRAM parameters (per-core shards, pre-tiled) ----------
    # q-path weights
    wqT_d = nc.dram_tensor("wqT", [128, 16, SH], fp8, kind="ExternalInput")
    wqmC_d = nc.dram_tensor("wqmC", [128, 2, QK], fp8, kind="ExternalInput")
    wkmC_d = nc.dram_tensor("wkmC", [128, 16, SH], fp8, kind="ExternalInput")
    wk_d = nc.dram_tensor("wk", [128, 2, VO], fp8, kind="ExternalInput")
    # keys + k-layout tables
    ck_d = nc.dram_tensor("ck", [128, 8, S_LOC], bf16, kind="ExternalInput")
    sk_d = nc.dram_tensor("sk", [128, 8, S_LOC], bf16, kind="ExternalInput")
    keysT_d = nc.dram_tensor("keysT", [128, 8, 2, S_LOC], bf16, kind="ExternalInput")
    # states + s-layout tables
    states_d = nc.dram_tensor("states", [128, 8, VO], bf16, kind="ExternalInput")
    # epilogue weights
    wvT_d = nc.dram_tensor("wvT", [128, 16, SH], fp8, kind="ExternalInput")
    wvm_d = nc.dram_tensor("wvm", [128, 2, VO], fp8, kind="ExternalInput")
    woT_d = nc.dram_tensor("woT", [128, 16, SH], fp8, kind="ExternalInput")
    # small stuff
    xq_d = nc.dram_tensor("xq", [128, 16], bf16, kind="ExternalInput")
    ib16_d = nc.dram_tensor("ib16", [16, 16], bf16, kind="ExternalInput")
    ibn16_d = nc.dram_tensor("ibn16", [16, 16], bf16, kind="ExternalInput")
    if16_d = nc.dram_tensor("if16", [16, 16], f32, kind="ExternalInput")
    xo_d = nc.dram_tensor("xo", [1, SH], f32, kind="ExternalInput")
    out_d = nc.dram_tensor("out", [1, SH], f32, kind="ExternalOutput")
    DEBUG = _cache.get("debug", False)
    if DEBUG:
        dqt_d = nc.dram_tensor("dbg_qt", [128, 16 * NUM_HEADS], f32, kind="ExternalOutput")
        dw_d = nc.dram_tensor("dbg_w", [NUM_HEADS, S_LOC], f32, kind="ExternalOutput")
        du_d = nc.dram_tensor("dbg_u", [128, 16 * NUM_HEADS + 1], f32, kind="ExternalOutput")
        dat_d = nc.dram_tensor("dbg_at", [DQ, NUM_HEADS], f32, kind="ExternalOutput")
        dz_d = nc.dram_tensor("dbg_z", [NUM_HEADS, SH], f32, kind="ExternalOutput")
        dwt_d = nc.dram_tensor("dbg_wt", [128, 8, NUM_HEADS], f32, kind="ExternalOutput")
        dwtn_d = nc.dram_tensor("dbg_wtn", [128, 8, NUM_HEADS], f32, kind="ExternalOutput")
        dqn_d = nc.dram_tensor("dbg_qn", [128, 16 * NUM_HEADS], f32, kind="ExternalOutput")

    RG = [list(range(NC))]
    SCALE = 1.0 / float(np.sqrt(DQ))

    with tile.TileContext(nc) as tc:
        with (
            tc.tile_pool(name="wts", bufs=4) as wts,
            tc.tile_pool(name="tabs", bufs=1) as tabs,
            tc.tile_pool(name="kbuf", bufs=3) as kbuf,
            tc.tile_pool(name="kp", bufs=24) as kp,
            tc.tile_pool(name="sbuf_s", bufs=2) as sbuf_s,
            tc.tile_pool(name="sp", bufs=16) as sp,
            tc.tile_pool(name="small", bufs=1) as small,
            tc.tile_pool(name="psL", bufs=2, space=PSUM) as psL,
            tc.tile_pool(name="psU", bufs=4, space=PSUM) as psU,
            tc.tile_pool(name="psS", bufs=2, space=PSUM) as psS,
            tc.tile_pool(name="dram", bufs=1, space="DRAM") as dram,
        ):
            # ---------------- collective bounce buffers (DRAM) --------------
            bqh_in = dram.tile([128, NUM_HEADS], f32)
            bqh_out = dram.tile([128, NUM_HEADS], f32)
            bqt_in = dram.tile([128, 16 * NUM_HEADS], bf16)
            bqt_out = dram.tile([128, 16 * NUM_HEADS], bf16)
            bu_in = dram.tile([128, 16 * NUM_HEADS + 1], f32)
            bu_out = dram.tile([128, 16 * NUM_HEADS + 1], f32)
            bat_in = dram.tile([DQ, NUM_HEADS], f32)
            bat_out = dram.tile([DQ, NUM_HEADS], f32)

            # ---------------- SBUF tiles ------------------------------------
            # q-path weights (rotate through 4 slots shared with epilogue wts)
            wqT_sb = wts.tile([128, 16, SH], fp8, tag="w8k")
            wqmC_sb = wts.tile([128, 2, QK], fp8, tag="w8k")
            wkmC_sb = wts.tile([128, 16, SH], fp8, tag="w8k")
            wk_sb = wts.tile([128, 2, VO], fp8, tag="w8k")

            ck_sb = tabs.tile([128, 8, S_LOC], bf16, tag="ck")
            sk_sb = tabs.tile([128, 8, S_LOC], bf16, tag="sk")
            csx_sb = tabs.tile([128, 8, 8, 128], bf16, tag="cs")
            ssx_sb = tabs.tile([128, 8, 8, 128], bf16, tag="ss")

            x_sb = small.tile([128, 16], bf16, tag="x")
            ib16_sb = small.tile([16, 16], bf16, tag="ib16")
            ibn16_sb = small.tile([16, 16], bf16, tag="ibn16")
            if16_sb = small.tile([16, 16], f32, tag="if16")
            xo_sb = small.tile([1, SH], f32, tag="xo")

            qT_sb = small.tile([128, 2], bf16, tag="qT")
            qhTp_sb = small.tile([128, NUM_HEADS], f32, tag="qhTp")
            qhT_sb = small.tile([128, NUM_HEADS], bf16, tag="qhT")
            tmpT_sb = small.tile([128, 2, NUM_HEADS], bf16, tag="tmpT")
            qtp_sb = small.tile([128, 16, NUM_HEADS], bf16, tag="qtp")
            qtilT_sb = small.tile([128, 16, NUM_HEADS], bf16, tag="qtilT")
            qtilN_sb = small.tile([128, 16, NUM_HEADS], bf16, tag="qtilN")

            w_sb = small.tile([NUM_HEADS, S_LOC], bf16, tag="w")
            l0_sb = small.tile([NUM_HEADS, 1], f32, tag="l0")
            l1_sb = small.tile([NUM_HEADS, 1], f32, tag="l1")
            lp_sb = small.tile([NUM_HEADS, 1], f32, tag="lp")
            wT_sb = small.tile([128, 8, NUM_HEADS], bf16, tag="wT")
            wTn_sb = small.tile([128, 8, NUM_HEADS], bf16, tag="wTn")

            u_sb = small.tile([NUM_HEADS, VO], f32, tag="u")
            uT_sb = small.tile([128, 16, NUM_HEADS], f32, tag="uT")
            uT_bf = small.tile([128, 16, NUM_HEADS], bf16, tag="uTb")
            l_sb = small.tile([NUM_HEADS, 1], f32, tag="l")
            rl_sb = small.tile([NUM_HEADS, 1], f32, tag="rl")
            z_sb = small.tile([NUM_HEADS, SH], bf16, tag="z")
            zT_sb = small.tile([128, 2, NUM_HEADS], bf16, tag="zT")
            atT_sb = small.tile([128, NUM_HEADS], f32, tag="atT")
            atT_bf = small.tile([128, NUM_HEADS], bf16, tag="atTb")
            out_sb = small.tile([1, SH], f32, tag="out")

            # ================ sync-queue bulk DMA, strict priority ===========
            nc.sync.dma_start(wqT_sb[:], wqT_d[:, :, :])
            nc.sync.dma_start(wqmC_sb[:], wqmC_d[:, :, :])
            nc.sync.dma_start(wkmC_sb[:], wkmC_d[:, :, :])
            nc.sync.dma_start(wk_sb[:], wk_d[:, :, :])

            kt = []
            for ci in range(8):
                nc.sync.dma_start(ck_sb[:, ci, :], ck_d[:, ci, :])
                nc.sync.dma_start(sk_sb[:, ci, :], sk_d[:, ci, :])
                t = kbuf.tile([128, 2, S_LOC], bf16, tag="kt", name=f"kt{ci}")
                nc.sync.dma_start(t[:], keysT_d[:, ci, :, :])
                kt.append(t)

            st = []
            for sb in range(8):
                t = sbuf_s.tile([128, VO], bf16, tag="st", name=f"st{sb}")
                nc.sync.dma_start(t[:], states_d[:, sb, :])
                st.append(t)
            # derive s-layout tables from k-layout tables (SBUF->SBUF XBAR):
            # csx[p, ci, sb, j'] = table[s = sb*128+p, j = ci*128+j']
            for ci in range(8):
                nc.sync.dma_start_transpose(csx_sb[:, ci, :, :], ck_sb[:, ci, :])
                nc.sync.dma_start_transpose(ssx_sb[:, ci, :, :], sk_sb[:, ci, :])

            wvT_sb = wts.tile([128, 16, SH], fp8, tag="w8k")
            wvm_sb = wts.tile([128, 2, VO], fp8, tag="w8k")
            woT_sb = wts.tile([128, 16, SH], fp8, tag="w8k")
            nc.sync.dma_start(wvT_sb[:], wvT_d[:, :, :])
            nc.sync.dma_start(wvm_sb[:], wvm_d[:, :, :])
            nc.sync.dma_start(woT_sb[:], woT_d[:, :, :])

            # ================ scalar-queue small DMAs ========================
            nc.scalar.dma_start(x_sb[:], xq_d[:, :])
            nc.scalar.dma_start(ib16_sb[:], ib16_d[:, :])
            nc.scalar.dma_start(ibn16_sb[:], ibn16_d[:, :])
            nc.scalar.dma_start(if16_sb[:], if16_d[:, :])
            nc.scalar.dma_start(xo_sb[:], xo_d[:, :])

            # ================ q-path =========================================
            # qT = (x @ Wq.T)^T  (local output shard as [128, 2])
            for nc2 in range(2):
                qt_ps2 = psS.tile([128, 1], f32, tag="pS", name=f"qt_ps2_{nc2}")
                for kc in range(16):
                    nc.tensor.matmul(qt_ps2[:], wqT_sb[:, kc, nc2 * 128 : (nc2 + 1) * 128],
                                     x_sb[:, kc : kc + 1], start=(kc == 0), stop=(kc == 15))
                nc.scalar.activation(qT_sb[:, nc2 : nc2 + 1], qt_ps2[:], AF.Copy, scale=1.0 / 64)

            # qhT partial [d, h] = (q_shard @ Wq_mha[:, shard].T)^T
            qhT_ps = psS.tile([128, NUM_HEADS], f32, tag="pS")
            for h in range(NUM_HEADS):
                for nc2 in range(2):
                    nc.tensor.matmul(qhT_ps[:, h : h + 1],
                                     wqmC_sb[:, nc2, h * 128 : (h + 1) * 128],
                                     qT_sb[:, nc2 : nc2 + 1],
                                     start=(nc2 == 0), stop=(nc2 == 1))
            nc.scalar.activation(qhTp_sb[:], qhT_ps[:], AF.Copy, scale=1.0 / 64)
            nc.gpsimd.dma_start(bqh_in[:], qhTp_sb[:])
            nc.gpsimd.collective_compute(
                "AllReduce", ALU.add, ins=[bqh_in[:].opt()], outs=[bqh_out[:].opt()],
                replica_groups=RG)
            nc.gpsimd.dma_start(qhT_sb[:], bqh_out[:, :])

            # tmpT[j, h] local j-shard
            tmpT_ps = [psS.tile([128, NUM_HEADS], f32, tag="pS", name=f"tmpT_ps{j}")
                       for j in range(2)]
            for h in range(NUM_HEADS):
                for jc in range(2):
                    nc.tensor.matmul(tmpT_ps[jc][:, h : h + 1],
                                     wkmC_sb[:, h, jc * 128 : (jc + 1) * 128],
                                     qhT_sb[:, h : h + 1], start=True, stop=True)
            for jc in range(2):
                nc.scalar.activation(tmpT_sb[:, jc, :], tmpT_ps[jc][:], AF.Copy, scale=1.0 / 64)

            # qtilT partial = Wk_shard.T-contract
            for ic in range(16):
                qt_ps = psS.tile([128, NUM_HEADS], f32, tag="pS")
                for jc in range(2):
                    nc.tensor.matmul(qt_ps[:], wk_sb[:, jc, ic * 128 : (ic + 1) * 128],
                                     tmpT_sb[:, jc, :], start=(jc == 0), stop=(jc == 1))
                nc.scalar.activation(qtp_sb[:, ic, :], qt_ps[:], AF.Copy, scale=1.0 / 64)
            nc.gpsimd.dma_start(
                bqt_in[:, :].rearrange("p (ic h) -> p ic h", ic=16), qtp_sb[:])
            nc.gpsimd.collective_compute(
                "AllReduce", ALU.add, ins=[bqt_in[:].opt()], outs=[bqt_out[:].opt()],
                replica_groups=RG)
            nc.gpsimd.dma_start(
                qtilT_sb[:], bqt_out[:, :].rearrange("p (ic h) -> p ic h", ic=16))
            # negated copy (supplies the "-" of rope's first-half combine)
            nc.scalar.activation(qtilN_sb[:], qtilT_sb[:], AF.Copy, scale=-1.0)

            # ================ keys: products + logits ========================
            # pair ci covers j-chunks (ci, ci+8):
            #   a = keys[:, j=ci*128+p],  b = keys[:, j=1024+ci*128+p]
            #   roped_a = a*ck - b*sk   (lhsT qtil[ci] / qtilN[ci])
            #   roped_b = b*ck + a*sk   (lhsT qtil[ci+8])
            lg_ps = [psL.tile([NUM_HEADS, 512], f32, tag="pL", name=f"lg{sc}")
                     for sc in range(2)]
            for ci in range(8):
                a = kt[ci][:, 0, :]
                b = kt[ci][:, 1, :]
                t1 = kp.tile([128, S_LOC], bf16, tag="kp", name=f"t1_{ci}")
                t2 = kp.tile([128, S_LOC], bf16, tag="kp", name=f"t2_{ci}")
                t3 = kp.tile([128, S_LOC], bf16, tag="kp", name=f"t3_{ci}")
                t4 = kp.tile([128, S_LOC], bf16, tag="kp", name=f"t4_{ci}")
                nc.vector.tensor_mul(t1[:], a, ck_sb[:, ci, :])
                nc.vector.tensor_mul(t2[:], b, sk_sb[:, ci, :])
                nc.vector.tensor_mul(t3[:], b, ck_sb[:, ci, :])
                nc.vector.tensor_mul(t4[:], a, sk_sb[:, ci, :])
                prods = [(t1, qtilT_sb[:, ci, :]), (t2, qtilN_sb[:, ci, :]),
                         (t3, qtilT_sb[:, ci + 8, :]), (t4, qtilT_sb[:, ci + 8, :])]
                for pi, (t, lhsT) in enumerate(prods):
                    for sc in range(2):
                        nc.tensor.matmul(lg_ps[sc][:], lhsT,
                                         t[:, sc * 512 : (sc + 1) * 512],
                                         start=(ci == 0 and pi == 0),
                                         stop=(ci == 7 and pi == 3))

            # ================ softmax (no max-subtraction) ===================
            nc.scalar.activation(w_sb[:, 0:512], lg_ps[0][:], AF.Exp,
                                 scale=SCALE, accum_out=l0_sb[:])
            nc.scalar.activation(w_sb[:, 512:1024], lg_ps[1][:], AF.Exp,
                                 scale=SCALE, accum_out=l1_sb[:])

            # wT (+ negated) via PE transpose: [16,128] slices -> [128,16]
            for sb in range(8):
                tr_ps = psS.tile([128, NUM_HEADS], bf16, tag="pS")
                nc.tensor.transpose(tr_ps[:], w_sb[:, sb * 128 : (sb + 1) * 128],
                                    ib16_sb[:, :])
                nc.scalar.activation(wT_sb[:, sb, :], tr_ps[:], AF.Copy)
                nc.scalar.activation(wTn_sb[:, sb, :], tr_ps[:], AF.Copy, scale=-1.0)

            # ================ states: products + u ===========================
            #   st1 = states[:, 0:1024], st2 = states[:, 1024:2048]
            #   u_lo += wT.T@(st1*cs) + wTn.T@(st2*ss)
            #   u_hi += wT.T@(st2*cs) + wT.T @(st1*ss)
            u_ps = [psU.tile([NUM_HEADS, 512], f32, tag="pU", name=f"u_ps{i}")
                    for i in range(4)]
            for sb in range(8):
                st1 = st[sb][:, 0:HALF]
                st2 = st[sb][:, HALF:VO]
                p1 = sp.tile([128, HALF], bf16, tag="sp", name=f"p1_{sb}")
                p2 = sp.tile([128, HALF], bf16, tag="sp", name=f"p2_{sb}")
                p3 = sp.tile([128, HALF], bf16, tag="sp", name=f"p3_{sb}")
                p4 = sp.tile([128, HALF], bf16, tag="sp", name=f"p4_{sb}")
                r3 = lambda ap: ap.rearrange("p (a b) -> p a b", a=8)
                nc.vector.tensor_mul(r3(p1[:]), r3(st1), csx_sb[:, :, sb, :])
                nc.gpsimd.tensor_mul(r3(p2[:]), r3(st2), ssx_sb[:, :, sb, :])
                nc.vector.tensor_mul(r3(p3[:]), r3(st2), csx_sb[:, :, sb, :])
                nc.vector.tensor_mul(r3(p4[:]), r3(st1), ssx_sb[:, :, sb, :])
                # banks 0/1 accumulate p1 then p2 per sb; banks 2/3 p3 then p4
                chunks = [(0, p1, wT_sb), (0, p2, wTn_sb), (2, p3, wT_sb), (2, p4, wT_sb)]
                for pi, (base, t, wtiles) in enumerate(chunks):
                    for nch in range(2):
                        nc.tensor.matmul(u_ps[base + nch][:], wtiles[:, sb, :],
                                         t[:, nch * 512 : (nch + 1) * 512],
                                         start=(sb == 0 and pi in (0, 2)),
                                         stop=(sb == 7 and pi in (1, 3)))

            # ================ u epilogue =====================================
            for nch in range(4):
                nc.scalar.activation(u_sb[:, nch * 512 : (nch + 1) * 512],
                                     u_ps[nch][:], AF.Copy)
            for ic in range(16):
                tr_ps = psS.tile([128, NUM_HEADS], f32, tag="pS")
                nc.tensor.transpose(tr_ps[:], u_sb[:, ic * 128 : (ic + 1) * 128],
                                    if16_sb[:, :])
                nc.scalar.activation(uT_sb[:, ic, :], tr_ps[:], AF.Copy)
            nc.vector.tensor_add(lp_sb[:], l0_sb[:], l1_sb[:])
            nc.gpsimd.dma_start(
                bu_in[:, 0:256].rearrange("p (ic h) -> p ic h", ic=16), uT_sb[:])
            nc.gpsimd.dma_start(bu_in[0:NUM_HEADS, 256:257], lp_sb[:])
            nc.gpsimd.collective_compute(
                "AllReduce", ALU.add, ins=[bu_in[:].opt()], outs=[bu_out[:].opt()],
                replica_groups=RG)
            nc.gpsimd.dma_start(
                uT_bf[:], bu_out[:, 0:256].rearrange("p (ic h) -> p ic h", ic=16))
            nc.gpsimd.dma_start(l_sb[:], bu_out[0:NUM_HEADS, 256:257])
            nc.vector.tensor_scalar_mul(l_sb[:], l_sb[:], 64.0)
            nc.vector.reciprocal(rl_sb[:], l_sb[:])

            # ================ tail: z, attn, out =============================
            z_ps = psS.tile([NUM_HEADS, SH], f32, tag="pS")
            for ic in range(16):
                nc.tensor.matmul(z_ps[:], uT_bf[:, ic, :], wvT_sb[:, ic, :],
                                 start=(ic == 0), stop=(ic == 15))
            nc.scalar.activation(z_sb[:], z_ps[:], AF.Copy, scale=rl_sb[:])

            for jc in range(2):
                tr_ps = psS.tile([128, NUM_HEADS], bf16, tag="pS")
                nc.tensor.transpose(tr_ps[:], z_sb[:, jc * 128 : (jc + 1) * 128],
                                    ib16_sb[:, :])
                nc.scalar.activation(zT_sb[:, jc, :], tr_ps[:], AF.Copy)

            at_ps = psS.tile([128, NUM_HEADS], f32, tag="pS")
            for h in range(NUM_HEADS):
                for jc in range(2):
                    nc.tensor.matmul(at_ps[:, h : h + 1],
                                     wvm_sb[:, jc, h * 128 : (h + 1) * 128],
                                     zT_sb[:, jc, h : h + 1],
                                     start=(jc == 0), stop=(jc == 1))
            nc.scalar.activation(atT_sb[:], at_ps[:], AF.Copy, scale=1.0 / 64)
            nc.gpsimd.dma_start(bat_in[:], atT_sb[:])
            nc.gpsimd.collective_compute(
                "AllReduce", ALU.add, ins=[bat_in[:].opt()], outs=[bat_out[:].opt()],
                replica_groups=RG)
            nc.gpsimd.dma_start(atT_bf[:], bat_out[:, :])

            if DEBUG:
                nc.gpsimd.dma_start(dqt_d[:, :], bqt_out[:, :])
                nc.gpsimd.dma_start(dqn_d[:, :].rearrange("p (ic h) -> p ic h", ic=16), qtilN_sb[:])
                nc.gpsimd.dma_start(dw_d[:, :], w_sb[:])
                nc.gpsimd.dma_start(dwt_d[:, :, :], wT_sb[:])
                nc.gpsimd.dma_start(dwtn_d[:, :, :], wTn_sb[:])
                nc.gpsimd.dma_start(du_d[:, :], bu_out[:, :])
                nc.gpsimd.dma_start(dz_d[:, :], z_sb[:])
                nc.gpsimd.dma_start(dat_d[:, :], bat_out[:, :])

            o_ps = psS.tile([1, SH], f32, tag="pS")
            for h in range(NUM_HEADS):
                nc.tensor.matmul(o_ps[:], atT_bf[:, h : h + 1], woT_sb[:, h, :],
                                 start=(h == 0), stop=(h == NUM_HEADS - 1))
            nc.vector.scalar_tensor_tensor(out_sb[:], o_ps[:], 1.0 / 64, xo_sb[:],
                                           mybir.AluOpType.mult, mybir.AluOpType.add)
            nc.gpsimd.dma_start(out_d[:, :], out_sb[:])

    nc.compile()
    return nc


def _tables():
    # mimic reference: f32 angles, f32 cos/sin
    freqs = 1.0 / (ROPE_THETA ** (np.arange(HALF, dtype=np.float32) * 2.0 / VO))
    ang = np.outer(np.arange(S, dtype=np.float32), freqs).astype(np.float32)  # (S, half)
    return np.cos(ang), np.sin(ang)


def _tile_rows(a, p=128):
    """[R, C] -> [p, R//p, C] with row index = t*p + lane."""
    r, c = a.shape
    return np.ascontiguousarray(a.reshape(r // p, p, c).transpose(1, 0, 2))


def kernel(x, keys, states, Wq, Wk, Wv, Wq_mha, Wk_mha, Wv_mha, Wo):
    from concourse import bass_utils

    if "nc" not in _cache:
        _cache["nc"] = _build()
    nc = _cache["nc"]

    x = np.asarray(x, np.float32)
    keys = np.asarray(keys, np.float32)
    states = np.asarray(states, np.float32)
    cos_t, sin_t = _tables()

    ib = np.eye(16, dtype=np.float32)
    in_maps = []
    for c in range(NC):
        rs = slice(c * SH, (c + 1) * SH)
        sq = slice(c * S_LOC, (c + 1) * S_LOC)
        cosc = cos_t[sq]            # (1024, 1024) [s_loc, j]
        sinc = sin_t[sq]
        kT = keys[sq].T             # (2048, 1024) [j, s_loc]
        # pre-paired keys: [p, pair, half, s]
        kp = np.ascontiguousarray(
            kT.reshape(2, 8, 128, S_LOC).transpose(2, 1, 0, 3))
        m = {
            "wqT": _tile_rows(Wq[rs].T * WSCALE).astype(F8E3),
            "wqmC": _tile_rows(Wq_mha[:, rs].T * WSCALE).astype(F8E3),
            "wkmC": _tile_rows(Wk_mha[:, rs] * WSCALE).astype(F8E3),
            "wk": _tile_rows(Wk[rs] * WSCALE).astype(F8E3),
            "ck": _tile_rows(np.ascontiguousarray(cosc.T)).astype(BF16),
            "sk": _tile_rows(np.ascontiguousarray(sinc.T)).astype(BF16),
            "keysT": kp.astype(BF16),
            "states": _tile_rows(states[sq]).astype(BF16),
            "wvT": _tile_rows(Wv[rs].T * WSCALE).astype(F8E3),
            "wvm": _tile_rows(Wv_mha[:, rs].T * WSCALE).astype(F8E3),
            "woT": _tile_rows(Wo[rs].T * WSCALE).astype(F8E3),
            "xq": np.ascontiguousarray(x.reshape(16, 128).T).astype(BF16),
            "ib16": ib.astype(BF16),
            "ibn16": (-ib).astype(BF16),
            "if16": ib,
            "xo": np.ascontiguousarray(x[rs])[None, :],
        }
        in_maps.append(m)

    global _last_in_maps, _last_res
    _last_in_maps = in_maps
    res = bass_utils.run_bass_kernel_spmd(nc, in_maps, core_ids=list(range(NC)))
    _last_res = res
    out = np.concatenate([np.asarray(res.results[c]["out"]).reshape(-1) for c in range(NC)])
    return out[None, :].astype(np.float32)
